# revision 43
# baseline (speedup 1.0000x reference)
"""Trainium2 Bass kernel for spherical deep GMM classifier (DGMMC).

Reference computation (B=8192, D=1024, C=128 classes, K=8 comps, N=C*K=1024):
    bw = clip(bandwidths, 1e-3, 100); a = 1/bw
    log_prob[b,n] = -0.5*(D*log(2pi) + D*log(bw[n]) + sq_dist[b,n]/bw[n])
    log_prob += log_softmax(weights.reshape(C,K),1).reshape(N)
    lse1[b,c]  = LSE_k(log_prob[b,c*K+k]) + log_softmax(priors)[c]
    out[b,c]   = lse1[b,c] - LSE_c(lse1[b,c])

Strategy: data-parallel over batch across 8 cores.  All per-component affine
terms are folded into an augmented GEMM assembled on the host (tiny prep):
    xT_aug     = [x.T; xsq_h; xsq_h; xsq_l; 1; 1]            (D+5, B)
    meansT_aug = [means.T * a; ah_h; ah_l; ah_h; c_h; c_l]   (D+5, N)
with ah = -0.5*a and c[n] = -0.5*(D*log(2pi)+D*log(bw)+m_sq*a)+log_w+log_prior,
each large-magnitude augmented row split into hi/lo fp16 pairs so the fp16
GEMM keeps ~22-bit precision on those rank-2 terms.  PSUM of the GEMM then
holds log_prob (incl. prior) directly; the device does the grouped K=8 LSE
and the row LSE over C.
"""

import math

import numpy as np

B, D, C, K = 8192, 1024, 128, 8
N = C * K
NCORES = 8
BLOC = B // NCORES  # rows per core
P = 128
NAUG = 5  # augmented rows (split fp16 rank-2 correction)
AUG = D + NAUG
NFULL = D // P  # full 128-row contraction chunks
LOG_2PI = math.log(2.0 * math.pi)

GEMM_DTYPE = "float16"

_CACHE: dict = {}


def _build_nc(gemm_dtype: str, cfg=None):
    import concourse.bacc as bacc
    import concourse.mybir as mybir
    import concourse.tile as tile
    from concourse.tile import add_dep_helper

    defaults = dict(
        psum_split="full",   # "full": one [P,1024] tile/bt; "half": two [P,512]
        gsum_engine="gpsimd",  # or "vector"
        final_engine="act",    # or "vector"
        bufs_work=4,
        bufs_small=6,
        psum_bufs=None,
        epi_pieces=1,  # epilogue processed in N//pieces-wide strips
    )
    cfg = {**defaults, **(cfg or {})}

    f32 = mybir.dt.float32
    gdt = getattr(mybir.dt, gemm_dtype)

    nc = bacc.Bacc(None, target_bir_lowering=False)
    # xt is host-prepacked into per-b-tile stripes already in SBUF layout:
    # [bt, p, chunk, col] with the augmented rows as chunk NFULL (zero-padded
    # partitions 5..127), so each b-tile's stationary operand is one
    # contiguous full-bandwidth DMA.
    xt = nc.dram_tensor(
        "xt", [BLOC // P, P, NFULL + 1, P], gdt, kind="ExternalInput"
    )
    mt = nc.dram_tensor("mt", [AUG, N], gdt, kind="ExternalInput")
    out = nc.dram_tensor("out", [BLOC, C], f32, kind="ExternalOutput")

    NB = BLOC // P  # number of 128-row batch tiles per core
    G = N // K  # groups (= classes = 128)
    NH = N // 512
    half = cfg["psum_split"] == "half"
    psum_bufs = cfg["psum_bufs"] or (8 if half else 4)

    with tile.TileContext(nc) as tc:
        with (
            tc.tile_pool(name="resident", bufs=1) as resident,
            tc.tile_pool(name="work", bufs=cfg["bufs_work"]) as work,
            tc.tile_pool(name="small", bufs=cfg["bufs_small"]) as small,
            tc.tile_pool(name="psum", bufs=psum_bufs, space="PSUM") as psum_pool,
        ):
            xt_sb = resident.tile([P, NB, NFULL + 1, P], gdt)
            mt_sb = resident.tile([P, NFULL + 1, N], gdt)
            # PE warm-up: dummy matmuls with no DMA dependency run during the
            # input-load head so the HAM clock gate reaches 8/8 before the
            # real matmuls start (~3.4us of sustained PE activity needed).
            wu_src = resident.tile([P, 512], gdt, tag="wu_src")
            nc.vector.memset(wu_src, 0.0)
            wu_ps = psum_pool.tile([P, 512], f32, tag="ps", name="wups")
            for _ in range(13):
                nc.tensor.matmul(
                    wu_ps, wu_src[:, 0:P], wu_src, start=True, stop=True
                )
            # load order: tiny mt augmented rows, b-tile 0's xt stripe, then
            # mt chunk-wise (paces the matmul stream), then remaining stripes
            nc.sync.dma_start(mt_sb[0:NAUG, NFULL, :], mt[D : D + NAUG])
            nc.sync.dma_start(xt_sb[:, 0], xt[0])
            for ch in range(NFULL):
                nc.sync.dma_start(mt_sb[:, ch, :], mt[ch * P : (ch + 1) * P, :])
            for bt2 in range(1, NB):
                nc.sync.dma_start(xt_sb[:, bt2], xt[bt2])

            last_mm_by_bt = []
            for bt in range(NB):
                bsl = slice(bt * P, (bt + 1) * P)
                # last tile: two staggered half-N accumulation groups so the
                # h0 epilogue chain overlaps the h1 matmuls, leaving only a
                # half-width chain after the PE stream ends
                stag = bool(cfg.get("tail_stagger")) and bt == NB - 1
                if half or stag:
                    pss = [
                        psum_pool.tile([P, 512], f32, tag="ps", name=f"ps{h}")
                        for h in range(NH)
                    ]
                else:
                    ps = psum_pool.tile([P, N], f32, tag="ps")
                    pss = [ps[:, h * 512 : (h + 1) * 512] for h in range(NH)]
                if stag:
                    prev_in_tile = None
                    for h in range(NH):
                        for ch in range(NFULL + 1):
                            kp = P if ch < NFULL else NAUG
                            mmi = nc.tensor.matmul(
                                pss[h],
                                xt_sb[0:kp, bt, ch, :],
                                mt_sb[0:kp, ch, h * 512 : (h + 1) * 512],
                                start=(ch == 0),
                                stop=(ch == NFULL),
                            )
                            if ch == 0:
                                dep = (
                                    prev_in_tile
                                    if prev_in_tile is not None
                                    else last_mm_by_bt[bt - 1]
                                )
                                add_dep_helper(
                                    mmi.ins,
                                    dep.ins,
                                    sync=False,
                                    reason="staggered half-group order",
                                )
                        prev_in_tile = mmi
                else:
                    # augmented chunk first: its data arrives earliest (tiny
                    # DMA issued first), while ch NFULL-1 arrives last — so
                    # putting aug at the head leaves only ch NFULL-1's two
                    # matmuls between the last DMA arrival and group end
                    order = [NFULL] + list(range(NFULL))
                    for oi, ch in enumerate(order):
                        kp = P if ch < NFULL else NAUG
                        lhsT = xt_sb[0:kp, bt, ch, :]
                        for h in range(NH):
                            mmi = nc.tensor.matmul(
                                pss[h],
                                lhsT,
                                mt_sb[0:kp, ch, h * 512 : (h + 1) * 512],
                                start=(oi == 0),
                                stop=(oi == NFULL),
                            )
                            # serialize b-tile groups on PE so each group
                            # completes (and its epilogue starts) ASAP
                            if oi == 0 and h == 0 and bt >= 1:
                                add_dep_helper(
                                    mmi.ins,
                                    last_mm_by_bt[bt - 1].ins,
                                    sync=False,
                                    reason="group-sequential PE order",
                                )
                last_mm_by_bt.append(mmi)

                # --- grouped LSE over K=8, per strip so each strip's chain
                # starts as soon as its accumulation group ends ---
                npieces = cfg["epi_pieces"]
                if (cfg.get("tail_strips") and bt >= NB - 2) or stag:
                    npieces = 2
                W = N // npieces
                assert npieces == 2 or not (half or stag)
                lse1 = work.tile([P, G], f32, tag="lse1")
                gmaxt = work.tile([P, G], f32, tag="gmaxt")
                gsums = []
                for h in range(npieces):
                    gh = W // K  # groups in this strip
                    gslice = slice(h * gh, (h + 1) * gh)
                    if half or stag:
                        pvsrc = pss[h]
                    else:
                        pvsrc = ps[:, h * W : (h + 1) * W]
                    pv = pvsrc.rearrange("p (g k) -> p g k", k=K)
                    gmax = gmaxt[:, gslice]
                    nc.vector.tensor_reduce(
                        gmax, pv, axis=mybir.AxisListType.X, op=mybir.AluOpType.max
                    )
                    ei = work.tile([P, W], f32, tag=f"ei{h}")
                    nc.vector.tensor_tensor(
                        ei.rearrange("p (g k) -> p g k", k=K),
                        pv,
                        gmax[:, :, None].to_broadcast((P, gh, K)),
                        mybir.AluOpType.subtract,
                    )
                    nc.scalar.activation(ei, ei, mybir.ActivationFunctionType.Exp)
                    eiv = ei.rearrange("p (g k) -> p g k", k=K)
                    gsum = small.tile([P, gh], f32, tag=f"gsum{h}")
                    gsums.append(gsum)
                    if cfg["gsum_engine"] == "gpsimd":
                        # grouped sum via pairwise tree (SBUF-only engine)
                        t1 = small.tile([P, gh, K // 2], f32, tag=f"t1{h}")
                        nc.gpsimd.tensor_tensor(
                            t1, eiv[:, :, 0::2], eiv[:, :, 1::2],
                            mybir.AluOpType.add,
                        )
                        t2 = small.tile([P, gh, K // 4], f32, tag=f"t2{h}")
                        nc.gpsimd.tensor_tensor(
                            t2, t1[:, :, 0::2], t1[:, :, 1::2],
                            mybir.AluOpType.add,
                        )
                        nc.gpsimd.tensor_tensor(
                            gsum, t2[:, :, 0], t2[:, :, 1], mybir.AluOpType.add
                        )
                    else:
                        nc.vector.tensor_reduce(
                            gsum, eiv, axis=mybir.AxisListType.X,
                            op=mybir.AluOpType.add,
                        )
                    lseh = small.tile([P, gh], f32, tag=f"lseh{h}")
                    nc.scalar.activation(
                        lseh, gsum, mybir.ActivationFunctionType.Ln
                    )
                    nc.vector.tensor_add(lse1[:, gslice], lseh, gmax)

                # --- LSE over classes + normalize ---
                # shift = max_c gmax (off the critical chain; valid LSE shift
                # since max_c gmax <= max_c lse1 <= max_c gmax + ln K)
                nrmax = small.tile([P, 1], f32, tag="nrmax")
                nc.vector.tensor_reduce(
                    nrmax,
                    gmaxt,
                    axis=mybir.AxisListType.X,
                    op=mybir.AluOpType.max,
                    negate=True,
                )
                e2 = work.tile([P, C], f32, tag="e2")
                s2 = small.tile([P, 1], f32, tag="s2")
                nc.scalar.activation(
                    e2,
                    lse1,
                    mybir.ActivationFunctionType.Exp,
                    bias=nrmax,
                    accum_out=s2,
                )
                lnz = small.tile([P, 1], f32, tag="lnz")
                nc.scalar.activation(lnz, s2, mybir.ActivationFunctionType.Ln)
                denom_neg = small.tile([P, 1], f32, tag="denom_neg")
                nc.gpsimd.tensor_tensor(
                    denom_neg, nrmax, lnz, mybir.AluOpType.subtract
                )  # -(ln(s2) + rmax)
                ot = work.tile([P, C], f32, tag="ot")
                if cfg["final_engine"] == "act":
                    nc.scalar.activation(
                        ot, lse1, mybir.ActivationFunctionType.Identity,
                        bias=denom_neg,
                    )
                else:
                    denom = small.tile([P, 1], f32, tag="denom")
                    nc.gpsimd.tensor_scalar_mul(denom, denom_neg, -1.0)
                    nc.vector.tensor_scalar_sub(ot, lse1, denom)
                nc.sync.dma_start(out[bsl, :], ot)

    # Make Exp and Ln resolve to the single combined table set so the
    # table-load pass doesn't ping-pong two sets every b-tile.  Keys and
    # their order are preserved (act_func_set_id indexes this dict in
    # insertion order and must keep matching act_info.json); we only strip
    # Exp/Ln from every other set so the combined one is the unique choice.
    orig_tables = bacc.get_activation_tables

    def _exp_ln_combined(arch):
        t = orig_tables(arch)
        combined = "natural_log_exp_and_others"
        if combined not in t:
            return t
        strip = {
            mybir.ActivationFunctionType.Exp,
            mybir.ActivationFunctionType.Ln,
        }
        return {
            k: (v if k == combined else (set(v) - strip)) for k, v in t.items()
        }

    bacc.get_activation_tables = _exp_ln_combined
    try:
        nc.compile()
    finally:
        bacc.get_activation_tables = orig_tables
    return nc


def _split16(v):
    hi = v.astype(np.float16).astype(np.float64)
    lo = v - hi
    return hi, lo


def _host_prep(x, means, bandwidths, weights, priors):
    """Build augmented transposed fp16 operands."""
    x = np.asarray(x, dtype=np.float32)
    means = np.asarray(means, dtype=np.float32)

    bw = np.clip(np.asarray(bandwidths, dtype=np.float64), 0.001, 100.0)
    a = 1.0 / bw
    m_sq = np.einsum("nd,nd->n", means.astype(np.float64), means.astype(np.float64))
    w = np.asarray(weights, dtype=np.float64).reshape(C, K)
    log_w = (w - np.log(np.exp(w - w.max(1, keepdims=True)).sum(1, keepdims=True))
             - w.max(1, keepdims=True)).reshape(N)
    pr = np.asarray(priors, dtype=np.float64)
    log_pri = pr - (np.log(np.exp(pr - pr.max()).sum()) + pr.max())
    cvec = (
        -0.5 * (D * LOG_2PI + D * np.log(bw) + m_sq * a)
        + log_w
        + np.repeat(log_pri, K)
    )
    ah = -0.5 * a
    # Center the rank-2 correction terms: subtracting mean(ah)*xsq[b] and
    # mean(c) adds per-row constants to log_prob, which cancel in the final
    # LSE normalization.  Keeps the fp16 augmented rows small (exactly zero
    # for constant bandwidths) and in range for any bandwidth regime.
    ah = ah - ah.mean()
    cvec = cvec - cvec.mean()

    xsq = np.einsum("bd,bd->b", x.astype(np.float64), x.astype(np.float64))
    xsq_h, xsq_l = _split16(xsq)
    ah_h, ah_l = _split16(ah)
    c_h, c_l = _split16(cvec)
    ones = np.ones_like(xsq)

    xt_aug = np.empty((AUG, B), dtype=np.float16)
    xt_aug[0:D] = x.T.astype(np.float16)
    xt_aug[D + 0] = xsq_h
    xt_aug[D + 1] = xsq_h
    xt_aug[D + 2] = xsq_l
    xt_aug[D + 3] = ones
    xt_aug[D + 4] = ones

    # pack into per-core, per-b-tile stripes [core, bt, p, chunk, col]
    nbt = BLOC // P
    xt_pack = np.zeros((NCORES, nbt, P, NFULL + 1, P), dtype=np.float16)
    main = xt_aug[0:D].reshape(NFULL, P, NCORES, nbt, P)
    xt_pack[:, :, :, 0:NFULL, :] = main.transpose(2, 3, 1, 0, 4)
    aug = xt_aug[D:].reshape(NAUG, NCORES, nbt, P)
    xt_pack[:, :, 0:NAUG, NFULL, :] = aug.transpose(1, 2, 0, 3)

    mt_aug = np.empty((AUG, N), dtype=np.float16)
    mt_aug[0:D] = (means.T * a).astype(np.float16)
    mt_aug[D + 0] = ah_h
    mt_aug[D + 1] = ah_l
    mt_aug[D + 2] = ah_h
    mt_aug[D + 3] = c_h
    mt_aug[D + 4] = c_l
    return xt_pack, mt_aug


def _run(x, means, bandwidths, weights, priors, trace=False, cfg=None):
    from concourse.bass_utils import run_bass_kernel_spmd

    key = (GEMM_DTYPE, tuple(sorted((cfg or {}).items())))
    if key not in _CACHE:
        _CACHE[key] = _build_nc(GEMM_DTYPE, cfg)
    nc = _CACHE[key]

    xt_pack, mt_aug = _host_prep(x, means, bandwidths, weights, priors)
    in_maps = [
        {"xt": np.ascontiguousarray(xt_pack[i]), "mt": mt_aug}
        for i in range(NCORES)
    ]
    res = run_bass_kernel_spmd(nc, in_maps, core_ids=list(range(NCORES)), trace=trace)
    out = np.concatenate([r["out"] for r in res.results], axis=0)
    return out, res


def kernel(x, means, bandwidths, weights, priors):
    out, _ = _run(x, means, bandwidths, weights, priors, trace=False)
    return out
